# revision 19
# baseline (speedup 1.0000x reference)
"""Trainium2 Bass kernel: RMSNorm + RoPE + causal attention + output projection.

Tensor-parallel over heads: 16 heads / 8 cores = 2 heads per core.
Each core computes a full [S, D] partial output (its heads' contribution to
the 'snh,dnh->sd' projection); the all-reduce is done host-side in the gather.

v2 design (fused streaming, bf16):
  - Host prep uploads the RMSNorm'd activations already transposed (h^T
    [D, S] bf16) plus bf16 weights, a pre-transposed wo^T, and fp32 RoPE
    tables.  No PE transposes and no DRAM scratch roundtrip remain.
  - Single pass over 8 q-chunks of 512: QK projections (+RoPE) append to
    per-head K^T in SBUF, V is projected directly into natural [t, hd]
    layout (ht-tile stationary), then causal attention for the chunk runs
    against all K/V tiles so far, followed by the inline output projection
    and a bf16 DMA of the partial output rows.
  - Scores are computed transposed (S^T[t, s]); softmax denominators via a
    ones-stationary matmul accumulating in PSUM; Z/PV lag scores/exp by 2
    (software pipeline) so a late exp never stalls the in-order PE queue.
  - PSUM: one pool, four 2-bank tag rings: main (proj pp / scores sp),
    aux (rope ps / V accum / outproj op), o (PV accum), z (denominator).
"""
import os
import sys
import types

import numpy as np

SEQ, D, NH, HD = 4096, 2048, 16, 128
NCORES = 8
HPC = NH // NCORES          # heads per core
M = HPC * HD                # per-core fused head dim (256)
EPS = 1e-6
ROPE_BASE = 10000.0
SM_SCALE = 1.0 / np.sqrt(HD)
CHUNK = 512                 # q-chunk
NCHUNK = SEQ // CHUNK       # 8
NT = SEQ // 128             # 32 s-tiles
DT = D // 128               # 16 d-tiles
LAG = 2


def _inject_ntff_hook():
    """Register the axon NTFF profiling hook (missing antenv.axon_hooks)."""
    if "antenv.axon_hooks" in sys.modules:
        return
    try:
        import antenv
        from trn_agent_boot.trn_boot import _ntff_profile_via_ctypes
    except ImportError:
        return
    holder = [None]
    mod = types.ModuleType("antenv.axon_hooks")
    mod.set_axon_ntff_profile_hook = lambda h: holder.__setitem__(0, h)
    mod.get_axon_ntff_profile_hook = lambda: holder[0]
    sys.modules["antenv.axon_hooks"] = mod
    antenv.axon_hooks = mod
    try:
        mod.set_axon_ntff_profile_hook(
            _ntff_profile_via_ctypes("/opt/axon/libaxon_pjrt.so"))
    except Exception:
        pass


def _build_nc():
    import concourse.bass as bass  # noqa: F401
    import concourse.mybir as mybir
    import concourse.tile as tile
    from concourse import bacc

    FP32 = mybir.dt.float32
    BF16 = mybir.dt.bfloat16
    AF = mybir.ActivationFunctionType
    ALU = mybir.AluOpType

    nc = bacc.Bacc(None, target_bir_lowering=False)

    ht_d = nc.declare_dram_parameter("ht", [128, DT, SEQ], BF16,
                                     isOutput=False)
    wq = nc.declare_dram_parameter("wq", [128, DT * M], BF16, isOutput=False)
    wk = nc.declare_dram_parameter("wk", [128, DT * M], BF16, isOutput=False)
    wv = nc.declare_dram_parameter("wv", [128, DT * M], BF16, isOutput=False)
    wot_d = nc.declare_dram_parameter("wot", [128, HPC * D], BF16,
                                      isOutput=False)
    cosd = nc.declare_dram_parameter("cosd", [128, SEQ], FP32, isOutput=False)
    sind = nc.declare_dram_parameter("sind", [128, SEQ], FP32, isOutput=False)
    tri = nc.declare_dram_parameter("tri", [128, 128], BF16, isOutput=False)
    ones = nc.declare_dram_parameter("ones", [128, 128], BF16, isOutput=False)
    out = nc.declare_dram_parameter("out", [SEQ, D], BF16, isOutput=True)

    with tile.TileContext(nc) as tc:
        with tc.tile_pool(name="consts", bufs=1) as consts, \
             tc.tile_pool(name="pht", bufs=2) as pht, \
             tc.tile_pool(name="pqt", bufs=4) as pqt, \
             tc.tile_pool(name="ppc", bufs=2) as ppc, \
             tc.tile_pool(name="ppt", bufs=6) as ppt, \
             tc.tile_pool(name="prz", bufs=2) as prz, \
             tc.tile_pool(name="pat", bufs=4) as pat, \
             tc.tile_pool(name="post", bufs=3) as post, \
             tc.tile_pool(name="pcs", bufs=2) as pcs, \
             tc.tile_pool(name="psum", bufs=2, space="PSUM") as psum:
            kt_sb = [consts.tile([128, SEQ], BF16, name=f"kt{h}")
                     for h in range(HPC)]
            # V natural, packed per t-tile: vn[p, jt*M + m] = V[jt*128+p, m]
            vn_sb = consts.tile([128, NT * M], BF16)

            def load_ht(c):
                cs = slice(c * CHUNK, (c + 1) * CHUNK)
                ht = pht.tile([128, DT, CHUNK], BF16, name="ht")
                for dt8 in range(2):
                    nc.sync.dma_start(
                        out=ht[:, dt8 * 8:(dt8 + 1) * 8, :],
                        in_=ht_d[:, dt8 * 8:(dt8 + 1) * 8, cs])
                return ht

            def load_cs(c):
                cs = slice(c * CHUNK, (c + 1) * CHUNK)
                sin_t = pcs.tile([128, CHUNK], FP32, name="sin_t", tag="sin")
                nc.sync.dma_start(out=sin_t[:], in_=sind[:, cs])
                cos_t = pcs.tile([128, CHUNK], FP32, name="cos_t", tag="cos")
                nc.sync.dma_start(out=cos_t[:], in_=cosd[:, cs])
                return cos_t, sin_t

            # startup: the sync queue issues only what the first projections
            # need (ht piece 0, wq); everything else issues in parallel from
            # the scalar engine's HWDGE queue.
            w_sbs = {k: consts.tile([128, DT, M], BF16, name=f"w{k}_sb")
                     for k in ("q", "k", "v")}
            cs0 = slice(0, CHUNK)
            ht = pht.tile([128, DT, CHUNK], BF16, name="ht")
            nc.sync.dma_start(out=ht[:, 0:8, :], in_=ht_d[:, 0:8, cs0])
            nc.sync.dma_start(
                out=w_sbs["q"][:].rearrange("p t m -> p (t m)"), in_=wq[:])
            nc.sync.dma_start(out=ht[:, 8:16, :], in_=ht_d[:, 8:16, cs0])
            cs_cur = load_cs(0)
            nc.scalar.dma_start(
                out=w_sbs["k"][:].rearrange("p t m -> p (t m)"), in_=wk[:])
            nc.scalar.dma_start(
                out=w_sbs["v"][:].rearrange("p t m -> p (t m)"), in_=wv[:])
            tri_sb = consts.tile([128, 128], BF16)
            nc.scalar.dma_start(out=tri_sb[:], in_=tri[:])
            ones_sb = consts.tile([128, 128], BF16)
            nc.scalar.dma_start(out=ones_sb[:], in_=ones[:])
            wot_sb = consts.tile([128, HPC * D], BF16)
            nc.scalar.dma_start(out=wot_sb[:], in_=wot_d[:])

            def outproj(ats, c):
                for st in range(4):
                    ost = post.tile([128, D], BF16, name="ost")
                    for dc in range(4):
                        op = psum.tile([128, CHUNK], FP32, name="op",
                                       tag="aux")
                        nc.tensor.matmul(
                            op[:],
                            ats[0][:, st * 128:(st + 1) * 128],
                            wot_sb[:, dc * CHUNK:(dc + 1) * CHUNK],
                            start=True, stop=False)
                        nc.tensor.matmul(
                            op[:],
                            ats[1][:, st * 128:(st + 1) * 128],
                            wot_sb[:, D + dc * CHUNK:D + (dc + 1) * CHUNK],
                            start=False, stop=True)
                        if dc % 2 == 0:
                            nc.scalar.activation(
                                ost[:, dc * CHUNK:(dc + 1) * CHUNK],
                                op[:], AF.Copy)
                        else:
                            nc.vector.tensor_copy(
                                ost[:, dc * CHUNK:(dc + 1) * CHUNK], op[:])
                    row = (c * 4 + st) * 128
                    nc.sync.dma_start(out=out[row:row + 128, :], in_=ost[:])

            ats_prev = None
            for c in range(NCHUNK):
                cs = slice(c * CHUNK, (c + 1) * CHUNK)
                cos_t, sin_t = cs_cur

                # ---- QK projections + RoPE
                qts = []
                for head in range(HPC):
                    for kind in ("q", "k"):
                        w_sb = w_sbs[kind]
                        pp = psum.tile([128, CHUNK], FP32, name="pp",
                                       tag="main")
                        for dt in range(DT):
                            nc.tensor.matmul(
                                pp[:],
                                w_sb[:, dt, head * HD:(head + 1) * HD],
                                ht[:, dt, :],
                                start=(dt == 0), stop=(dt == DT - 1))
                        ps = psum.tile([128, CHUNK], FP32, name="ps",
                                       tag="aux")
                        nc.vector.tensor_tensor(
                            out=ps[:], in0=pp[:], in1=sin_t[:],
                            op=ALU.mult)
                        pc = ppc.tile([128, CHUNK], FP32, name="pc")
                        nc.vector.tensor_tensor(
                            out=pc[:], in0=pp[:], in1=cos_t[:],
                            op=ALU.mult)
                        if kind == "q":
                            dst = pqt.tile([128, CHUNK], BF16,
                                           name=f"qt{head}")
                            qts.append(dst)
                            d0, d1 = dst[0:64, :], dst[64:128, :]
                        else:
                            d0 = kt_sb[head][0:64, cs]
                            d1 = kt_sb[head][64:128, cs]
                        nc.vector.tensor_tensor(
                            out=d0, in0=pc[0:64, :], in1=ps[64:128, :],
                            op=ALU.subtract)
                        nc.vector.tensor_tensor(
                            out=d1, in0=pc[64:128, :], in1=ps[0:64, :],
                            op=ALU.add)

                # ---- V projection directly into natural [t, m] layout
                for g in range(2):
                    pv = psum.tile([128, CHUNK], FP32, name="pv", tag="aux")
                    for half in range(2):
                        st = g * 2 + half
                        for dt in range(DT):
                            nc.tensor.matmul(
                                pv[:, half * M:(half + 1) * M],
                                ht[:, dt, st * 128:(st + 1) * 128],
                                w_sbs["v"][:, dt, :],
                                start=(dt == 0), stop=(dt == DT - 1))
                    t0 = (c * 4 + g * 2) * M
                    nc.vector.tensor_copy(vn_sb[:, t0:t0 + 2 * M], pv[:])

                # prefetch next chunk's h^T + rope slices while computing
                if c + 1 < NCHUNK:
                    ht = load_ht(c + 1)
                    cs_cur = load_cs(c + 1)

                # previous chunk's output projection here: its matmuls cover
                # the recip+normalize latency of this emission point's
                # preceding attention, and this chunk's attention covers the
                # at-normalize of chunk c-1.
                if ats_prev is not None:
                    outproj(ats_prev, c - 1)

                # ---- causal attention per head
                ats = []
                for head in range(HPC):
                    o_acc = psum.tile([128, CHUNK], FP32, name="o_acc",
                                      tag="o")
                    z_acc = psum.tile([128, CHUNK], FP32, name="z_acc",
                                      tag="z")
                    jmax = 4 * c + 3
                    pts = {}
                    for jj in range(jmax + 1 + LAG):
                        if jj <= jmax:
                            j = jj
                            r = j - 4 * c
                            off = max(r, 0) * 128
                            sp = psum.tile([128, CHUNK], FP32, name="sp",
                                           tag="main")
                            nc.tensor.matmul(
                                sp[:, off:],
                                kt_sb[head][:, j * 128:(j + 1) * 128],
                                qts[head][:, off:],
                                start=True, stop=True)
                            pt = ppt.tile([128, CHUNK], BF16, name="pt")
                            nc.scalar.activation(pt[:, off:], sp[:, off:],
                                                 AF.Exp,
                                                 scale=float(SM_SCALE))
                            if r >= 0:
                                nc.vector.tensor_tensor(
                                    out=pt[:, off:off + 128],
                                    in0=pt[:, off:off + 128],
                                    in1=tri_sb[:], op=ALU.mult)
                            pts[j] = (pt, off)
                        if jj >= LAG:
                            j = jj - LAG
                            pt, off = pts.pop(j)
                            nc.tensor.matmul(
                                z_acc[:, off:], ones_sb[:], pt[:, off:],
                                start=(j == 0), stop=(j == jmax),
                                skip_group_check=True)
                            nc.tensor.matmul(
                                o_acc[:, off:],
                                vn_sb[:, j * M + head * HD:
                                      j * M + (head + 1) * HD],
                                pt[:, off:],
                                start=(j == 0), stop=(j == jmax),
                                skip_group_check=True)
                    rz = prz.tile([128, CHUNK], FP32, name="rz")
                    nc.vector.reciprocal_approx_fast(out=rz[:], in_=z_acc[:])
                    at = pat.tile([128, CHUNK], BF16, name=f"at{head}")
                    nc.vector.tensor_tensor(out=at[:], in0=o_acc[:],
                                            in1=rz[:], op=ALU.mult)
                    ats.append(at)
                ats_prev = ats
            outproj(ats_prev, NCHUNK - 1)
    nc.finalize()
    return nc


def _host_prep(xs, norm_w, wq, wk, wv, wo):
    """Fold RMSNorm into h^T upload + weights; build rope tables; bf16."""
    import ml_dtypes
    BF = ml_dtypes.bfloat16

    x64 = xs.astype(np.float64)
    istd = 1.0 / np.sqrt((x64 * x64).mean(axis=1) + EPS)    # [S]
    ht = (x64 * istd[:, None]).T.astype(BF)                 # [D, S]
    # partition-major pack: ht_pm[p, dt, s] = ht[dt*128+p, s]
    ht_pm = np.ascontiguousarray(
        ht.reshape(DT, 128, SEQ).transpose(1, 0, 2))

    def pack_w(w):  # [D, M] -> [128, DT*M] partition-major
        return np.ascontiguousarray(
            w.reshape(DT, 128, M).transpose(1, 0, 2).reshape(128, DT * M))

    nw = norm_w.astype(np.float32)[:, None, None]
    perm = np.concatenate([np.arange(0, HD, 2), np.arange(1, HD, 2)])
    wq_p = (wq * nw)[:, :, perm]
    wk_p = (wk * nw)[:, :, perm]

    inv_freq = 1.0 / (ROPE_BASE ** (np.arange(0, HD, 2, dtype=np.float32) / HD))
    pos = np.arange(SEQ, dtype=np.float32)
    ang = pos[:, None] * inv_freq[None, :]          # [S, 64]
    cos_t = np.cos(ang).T.astype(np.float32)        # [64, S]
    sin_t = np.sin(ang).T.astype(np.float32)
    cosd = np.ascontiguousarray(np.concatenate([cos_t, cos_t], 0))
    sind = np.ascontiguousarray(np.concatenate([sin_t, sin_t], 0))

    tri = np.triu(np.ones((128, 128), dtype=np.float32)).astype(BF)
    onesm = np.ones((128, 128), dtype=BF)

    common = {
        "ht": ht_pm,
        "cosd": cosd,
        "sind": sind,
        "tri": np.ascontiguousarray(tri),
        "ones": onesm,
    }
    in_maps = []
    for core in range(NCORES):
        sl = slice(core * HPC, (core + 1) * HPC)
        wot = np.transpose(wo[:, sl, :], (2, 1, 0)).reshape(128, HPC * D)
        in_maps.append({
            **common,
            "wq": pack_w(wq_p[:, sl, :].reshape(D, M).astype(BF)),
            "wk": pack_w(wk_p[:, sl, :].reshape(D, M).astype(BF)),
            "wv": pack_w((wv * nw)[:, sl, :].reshape(D, M).astype(BF)),
            "wot": np.ascontiguousarray(wot.astype(BF)),
        })
    return in_maps


def kernel(xs, norm_w, wq, wk, wv, wo):
    trace = bool(int(os.environ.get("KERNEL_TRACE", "0")))
    if trace:
        _inject_ntff_hook()
    from concourse.bass_utils import run_bass_kernel_spmd

    nc = _build_nc()
    in_maps = _host_prep(np.asarray(xs), np.asarray(norm_w), np.asarray(wq),
                         np.asarray(wk), np.asarray(wv), np.asarray(wo))
    res = run_bass_kernel_spmd(nc, in_maps, core_ids=list(range(NCORES)),
                               trace=trace)
    if trace and res.exec_time_ns is not None:
        print(f"HW exec time: {res.exec_time_ns} ns")
    acc = np.zeros((SEQ, D), dtype=np.float64)
    for r in res.results:
        acc += r["out"].astype(np.float64)
    return acc.astype(np.float32)


if __name__ == "__main__":
    rng = np.random.default_rng(0)
    scale = 1.0 / np.sqrt(D)
    inputs = {
        "xs": rng.standard_normal((SEQ, D), dtype=np.float32),
        "norm_w": np.ones((D,), np.float32),
        "wq": rng.standard_normal((D, NH, HD), dtype=np.float32) * scale,
        "wk": rng.standard_normal((D, NH, HD), dtype=np.float32) * scale,
        "wv": rng.standard_normal((D, NH, HD), dtype=np.float32) * scale,
        "wo": rng.standard_normal((D, NH, HD), dtype=np.float32) * scale,
    }
    out = kernel(**inputs)
    print(out.shape, out.dtype, float(np.abs(out).max()))


# revision 20
# speedup vs baseline: 1.0274x; 1.0274x over previous
"""Trainium2 Bass kernel: RMSNorm + RoPE + causal attention + output projection.

Tensor-parallel over heads: 16 heads / 8 cores = 2 heads per core.
Each core computes a full [S, D] partial output (its heads' contribution to
the 'snh,dnh->sd' projection); the all-reduce is done host-side in the gather.

v2 design (fused streaming, bf16):
  - Host prep uploads the RMSNorm'd activations already transposed (h^T
    [D, S] bf16) plus bf16 weights, a pre-transposed wo^T, and fp32 RoPE
    tables.  No PE transposes and no DRAM scratch roundtrip remain.
  - Single pass over 8 q-chunks of 512: QK projections (+RoPE) append to
    per-head K^T in SBUF, V is projected directly into natural [t, hd]
    layout (ht-tile stationary), then causal attention for the chunk runs
    against all K/V tiles so far, followed by the inline output projection
    and a bf16 DMA of the partial output rows.
  - Scores are computed transposed (S^T[t, s]); softmax denominators via a
    ones-stationary matmul accumulating in PSUM; Z/PV lag scores/exp by 2
    (software pipeline) so a late exp never stalls the in-order PE queue.
  - PSUM: one pool, four 2-bank tag rings: main (proj pp / scores sp),
    aux (rope ps / V accum / outproj op), o (PV accum), z (denominator).
"""
import os
import sys
import types

import numpy as np

SEQ, D, NH, HD = 4096, 2048, 16, 128
NCORES = 8
HPC = NH // NCORES          # heads per core
M = HPC * HD                # per-core fused head dim (256)
EPS = 1e-6
ROPE_BASE = 10000.0
SM_SCALE = 1.0 / np.sqrt(HD)
CHUNK = 512                 # q-chunk
NCHUNK = SEQ // CHUNK       # 8
NT = SEQ // 128             # 32 s-tiles
DT = D // 128               # 16 d-tiles
LAG = 2


def _inject_ntff_hook():
    """Register the axon NTFF profiling hook (missing antenv.axon_hooks)."""
    if "antenv.axon_hooks" in sys.modules:
        return
    try:
        import antenv
        from trn_agent_boot.trn_boot import _ntff_profile_via_ctypes
    except ImportError:
        return
    holder = [None]
    mod = types.ModuleType("antenv.axon_hooks")
    mod.set_axon_ntff_profile_hook = lambda h: holder.__setitem__(0, h)
    mod.get_axon_ntff_profile_hook = lambda: holder[0]
    sys.modules["antenv.axon_hooks"] = mod
    antenv.axon_hooks = mod
    try:
        mod.set_axon_ntff_profile_hook(
            _ntff_profile_via_ctypes("/opt/axon/libaxon_pjrt.so"))
    except Exception:
        pass


def _build_nc():
    import concourse.bass as bass  # noqa: F401
    import concourse.mybir as mybir
    import concourse.tile as tile
    from concourse import bacc

    FP32 = mybir.dt.float32
    BF16 = mybir.dt.bfloat16
    AF = mybir.ActivationFunctionType
    ALU = mybir.AluOpType

    nc = bacc.Bacc(None, target_bir_lowering=False)

    ht_d = nc.declare_dram_parameter("ht", [128, DT, SEQ], BF16,
                                     isOutput=False)
    wq = nc.declare_dram_parameter("wq", [128, DT * M], BF16, isOutput=False)
    wk = nc.declare_dram_parameter("wk", [128, DT * M], BF16, isOutput=False)
    wv = nc.declare_dram_parameter("wv", [128, DT * M], BF16, isOutput=False)
    wot_d = nc.declare_dram_parameter("wot", [128, HPC * D], BF16,
                                      isOutput=False)
    cosd = nc.declare_dram_parameter("cosd", [128, SEQ], FP32, isOutput=False)
    sind = nc.declare_dram_parameter("sind", [128, SEQ], FP32, isOutput=False)
    tri = nc.declare_dram_parameter("tri", [128, 128], BF16, isOutput=False)
    ones = nc.declare_dram_parameter("ones", [128, 128], BF16, isOutput=False)
    out = nc.declare_dram_parameter("out", [SEQ, D], BF16, isOutput=True)

    with tile.TileContext(nc) as tc:
        with tc.tile_pool(name="consts", bufs=1) as consts, \
             tc.tile_pool(name="pht", bufs=2) as pht, \
             tc.tile_pool(name="pqt", bufs=4) as pqt, \
             tc.tile_pool(name="ppc", bufs=2) as ppc, \
             tc.tile_pool(name="ppt", bufs=6) as ppt, \
             tc.tile_pool(name="prz", bufs=2) as prz, \
             tc.tile_pool(name="pat", bufs=4) as pat, \
             tc.tile_pool(name="post", bufs=3) as post, \
             tc.tile_pool(name="pcs", bufs=2) as pcs, \
             tc.tile_pool(name="psum", bufs=2, space="PSUM") as psum:
            kt_sb = [consts.tile([128, SEQ], BF16, name=f"kt{h}")
                     for h in range(HPC)]
            # V natural, packed per t-tile: vn[p, jt*M + m] = V[jt*128+p, m]
            vn_sb = consts.tile([128, NT * M], BF16)

            def load_ht(c):
                cs = slice(c * CHUNK, (c + 1) * CHUNK)
                ht = pht.tile([128, DT, CHUNK], BF16, name="ht")
                for dt8 in range(2):
                    nc.sync.dma_start(
                        out=ht[:, dt8 * 8:(dt8 + 1) * 8, :],
                        in_=ht_d[:, dt8 * 8:(dt8 + 1) * 8, cs])
                return ht

            def load_cs(c):
                cs = slice(c * CHUNK, (c + 1) * CHUNK)
                sin_t = pcs.tile([128, CHUNK], FP32, name="sin_t", tag="sin")
                nc.sync.dma_start(out=sin_t[:], in_=sind[:, cs])
                cos_t = pcs.tile([128, CHUNK], FP32, name="cos_t", tag="cos")
                nc.sync.dma_start(out=cos_t[:], in_=cosd[:, cs])
                return cos_t, sin_t

            # startup: the sync queue issues only what the first projections
            # need (ht piece 0, wq); everything else issues in parallel from
            # the scalar engine's HWDGE queue.
            w_sbs = {k: consts.tile([128, DT, M], BF16, name=f"w{k}_sb")
                     for k in ("q", "k", "v")}
            cs0 = slice(0, CHUNK)
            ht = pht.tile([128, DT, CHUNK], BF16, name="ht")
            nc.sync.dma_start(out=ht[:, 0:8, :], in_=ht_d[:, 0:8, cs0])
            nc.sync.dma_start(
                out=w_sbs["q"][:, 0:8, :].rearrange("p t m -> p (t m)"),
                in_=wq[:, :8 * M])
            cs_cur = load_cs(0)
            nc.sync.dma_start(
                out=w_sbs["q"][:, 8:16, :].rearrange("p t m -> p (t m)"),
                in_=wq[:, 8 * M:])
            nc.sync.dma_start(out=ht[:, 8:16, :], in_=ht_d[:, 8:16, cs0])
            nc.sync.dma_start(
                out=w_sbs["k"][:].rearrange("p t m -> p (t m)"), in_=wk[:])
            nc.sync.dma_start(
                out=w_sbs["v"][:].rearrange("p t m -> p (t m)"), in_=wv[:])
            tri_sb = consts.tile([128, 128], BF16)
            nc.sync.dma_start(out=tri_sb[:], in_=tri[:])
            ones_sb = consts.tile([128, 128], BF16)
            nc.sync.dma_start(out=ones_sb[:], in_=ones[:])
            wot_sb = consts.tile([128, HPC * D], BF16)
            nc.sync.dma_start(out=wot_sb[:], in_=wot_d[:])

            def outproj(ats, c):
                for st in range(4):
                    ost = post.tile([128, D], BF16, name="ost")
                    for dc in range(4):
                        op = psum.tile([128, CHUNK], FP32, name="op",
                                       tag="aux")
                        nc.tensor.matmul(
                            op[:],
                            ats[0][:, st * 128:(st + 1) * 128],
                            wot_sb[:, dc * CHUNK:(dc + 1) * CHUNK],
                            start=True, stop=False)
                        nc.tensor.matmul(
                            op[:],
                            ats[1][:, st * 128:(st + 1) * 128],
                            wot_sb[:, D + dc * CHUNK:D + (dc + 1) * CHUNK],
                            start=False, stop=True)
                        if dc % 2 == 0:
                            nc.scalar.activation(
                                ost[:, dc * CHUNK:(dc + 1) * CHUNK],
                                op[:], AF.Copy)
                        else:
                            nc.vector.tensor_copy(
                                ost[:, dc * CHUNK:(dc + 1) * CHUNK], op[:])
                    row = (c * 4 + st) * 128
                    nc.sync.dma_start(out=out[row:row + 128, :], in_=ost[:])

            ats_prev = None
            for c in range(NCHUNK):
                cs = slice(c * CHUNK, (c + 1) * CHUNK)
                cos_t, sin_t = cs_cur

                # ---- QK projections + RoPE
                qts = []
                for head in range(HPC):
                    for kind in ("q", "k"):
                        w_sb = w_sbs[kind]
                        pp = psum.tile([128, CHUNK], FP32, name="pp",
                                       tag="main")
                        for dt in range(DT):
                            nc.tensor.matmul(
                                pp[:],
                                w_sb[:, dt, head * HD:(head + 1) * HD],
                                ht[:, dt, :],
                                start=(dt == 0), stop=(dt == DT - 1))
                        ps = psum.tile([128, CHUNK], FP32, name="ps",
                                       tag="aux")
                        nc.vector.tensor_tensor(
                            out=ps[:], in0=pp[:], in1=sin_t[:],
                            op=ALU.mult)
                        pc = ppc.tile([128, CHUNK], FP32, name="pc")
                        nc.vector.tensor_tensor(
                            out=pc[:], in0=pp[:], in1=cos_t[:],
                            op=ALU.mult)
                        if kind == "q":
                            dst = pqt.tile([128, CHUNK], BF16,
                                           name=f"qt{head}")
                            qts.append(dst)
                            d0, d1 = dst[0:64, :], dst[64:128, :]
                        else:
                            d0 = kt_sb[head][0:64, cs]
                            d1 = kt_sb[head][64:128, cs]
                        nc.vector.tensor_tensor(
                            out=d0, in0=pc[0:64, :], in1=ps[64:128, :],
                            op=ALU.subtract)
                        nc.vector.tensor_tensor(
                            out=d1, in0=pc[64:128, :], in1=ps[0:64, :],
                            op=ALU.add)

                # ---- V projection directly into natural [t, m] layout
                for g in range(2):
                    pv = psum.tile([128, CHUNK], FP32, name="pv", tag="aux")
                    for half in range(2):
                        st = g * 2 + half
                        for dt in range(DT):
                            nc.tensor.matmul(
                                pv[:, half * M:(half + 1) * M],
                                ht[:, dt, st * 128:(st + 1) * 128],
                                w_sbs["v"][:, dt, :],
                                start=(dt == 0), stop=(dt == DT - 1))
                    t0 = (c * 4 + g * 2) * M
                    nc.vector.tensor_copy(vn_sb[:, t0:t0 + 2 * M], pv[:])

                # prefetch next chunk's h^T + rope slices while computing
                if c + 1 < NCHUNK:
                    ht = load_ht(c + 1)
                    cs_cur = load_cs(c + 1)

                # previous chunk's output projection here: its matmuls cover
                # the recip+normalize latency of this emission point's
                # preceding attention, and this chunk's attention covers the
                # at-normalize of chunk c-1.
                if ats_prev is not None:
                    outproj(ats_prev, c - 1)

                # ---- causal attention per head
                ats = []
                for head in range(HPC):
                    o_acc = psum.tile([128, CHUNK], FP32, name="o_acc",
                                      tag="o")
                    z_acc = psum.tile([128, CHUNK], FP32, name="z_acc",
                                      tag="z")
                    jmax = 4 * c + 3
                    pts = {}
                    for jj in range(jmax + 1 + LAG):
                        if jj <= jmax:
                            j = jj
                            r = j - 4 * c
                            off = max(r, 0) * 128
                            sp = psum.tile([128, CHUNK], FP32, name="sp",
                                           tag="main")
                            nc.tensor.matmul(
                                sp[:, off:],
                                kt_sb[head][:, j * 128:(j + 1) * 128],
                                qts[head][:, off:],
                                start=True, stop=True)
                            pt = ppt.tile([128, CHUNK], BF16, name="pt")
                            nc.scalar.activation(pt[:, off:], sp[:, off:],
                                                 AF.Exp,
                                                 scale=float(SM_SCALE))
                            if r >= 0:
                                nc.vector.tensor_tensor(
                                    out=pt[:, off:off + 128],
                                    in0=pt[:, off:off + 128],
                                    in1=tri_sb[:], op=ALU.mult)
                            pts[j] = (pt, off)
                        if jj >= LAG:
                            j = jj - LAG
                            pt, off = pts.pop(j)
                            nc.tensor.matmul(
                                z_acc[:, off:], ones_sb[:], pt[:, off:],
                                start=(j == 0), stop=(j == jmax),
                                skip_group_check=True)
                            nc.tensor.matmul(
                                o_acc[:, off:],
                                vn_sb[:, j * M + head * HD:
                                      j * M + (head + 1) * HD],
                                pt[:, off:],
                                start=(j == 0), stop=(j == jmax),
                                skip_group_check=True)
                    rz = prz.tile([128, CHUNK], FP32, name="rz")
                    nc.vector.reciprocal_approx_fast(out=rz[:], in_=z_acc[:])
                    at = pat.tile([128, CHUNK], BF16, name=f"at{head}")
                    nc.vector.tensor_tensor(out=at[:], in0=o_acc[:],
                                            in1=rz[:], op=ALU.mult)
                    ats.append(at)
                ats_prev = ats
            outproj(ats_prev, NCHUNK - 1)
    nc.finalize()
    return nc


def _host_prep(xs, norm_w, wq, wk, wv, wo):
    """Fold RMSNorm into h^T upload + weights; build rope tables; bf16."""
    import ml_dtypes
    BF = ml_dtypes.bfloat16

    x64 = xs.astype(np.float64)
    istd = 1.0 / np.sqrt((x64 * x64).mean(axis=1) + EPS)    # [S]
    ht = (x64 * istd[:, None]).T.astype(BF)                 # [D, S]
    # partition-major pack: ht_pm[p, dt, s] = ht[dt*128+p, s]
    ht_pm = np.ascontiguousarray(
        ht.reshape(DT, 128, SEQ).transpose(1, 0, 2))

    def pack_w(w):  # [D, M] -> [128, DT*M] partition-major
        return np.ascontiguousarray(
            w.reshape(DT, 128, M).transpose(1, 0, 2).reshape(128, DT * M))

    nw = norm_w.astype(np.float32)[:, None, None]
    perm = np.concatenate([np.arange(0, HD, 2), np.arange(1, HD, 2)])
    wq_p = (wq * nw)[:, :, perm]
    wk_p = (wk * nw)[:, :, perm]

    inv_freq = 1.0 / (ROPE_BASE ** (np.arange(0, HD, 2, dtype=np.float32) / HD))
    pos = np.arange(SEQ, dtype=np.float32)
    ang = pos[:, None] * inv_freq[None, :]          # [S, 64]
    cos_t = np.cos(ang).T.astype(np.float32)        # [64, S]
    sin_t = np.sin(ang).T.astype(np.float32)
    cosd = np.ascontiguousarray(np.concatenate([cos_t, cos_t], 0))
    sind = np.ascontiguousarray(np.concatenate([sin_t, sin_t], 0))

    tri = np.triu(np.ones((128, 128), dtype=np.float32)).astype(BF)
    onesm = np.ones((128, 128), dtype=BF)

    common = {
        "ht": ht_pm,
        "cosd": cosd,
        "sind": sind,
        "tri": np.ascontiguousarray(tri),
        "ones": onesm,
    }
    in_maps = []
    for core in range(NCORES):
        sl = slice(core * HPC, (core + 1) * HPC)
        wot = np.transpose(wo[:, sl, :], (2, 1, 0)).reshape(128, HPC * D)
        in_maps.append({
            **common,
            "wq": pack_w(wq_p[:, sl, :].reshape(D, M).astype(BF)),
            "wk": pack_w(wk_p[:, sl, :].reshape(D, M).astype(BF)),
            "wv": pack_w((wv * nw)[:, sl, :].reshape(D, M).astype(BF)),
            "wot": np.ascontiguousarray(wot.astype(BF)),
        })
    return in_maps


def kernel(xs, norm_w, wq, wk, wv, wo):
    trace = bool(int(os.environ.get("KERNEL_TRACE", "0")))
    if trace:
        _inject_ntff_hook()
    from concourse.bass_utils import run_bass_kernel_spmd

    nc = _build_nc()
    in_maps = _host_prep(np.asarray(xs), np.asarray(norm_w), np.asarray(wq),
                         np.asarray(wk), np.asarray(wv), np.asarray(wo))
    res = run_bass_kernel_spmd(nc, in_maps, core_ids=list(range(NCORES)),
                               trace=trace)
    if trace and res.exec_time_ns is not None:
        print(f"HW exec time: {res.exec_time_ns} ns")
    acc = np.zeros((SEQ, D), dtype=np.float64)
    for r in res.results:
        acc += r["out"].astype(np.float64)
    return acc.astype(np.float32)


if __name__ == "__main__":
    rng = np.random.default_rng(0)
    scale = 1.0 / np.sqrt(D)
    inputs = {
        "xs": rng.standard_normal((SEQ, D), dtype=np.float32),
        "norm_w": np.ones((D,), np.float32),
        "wq": rng.standard_normal((D, NH, HD), dtype=np.float32) * scale,
        "wk": rng.standard_normal((D, NH, HD), dtype=np.float32) * scale,
        "wv": rng.standard_normal((D, NH, HD), dtype=np.float32) * scale,
        "wo": rng.standard_normal((D, NH, HD), dtype=np.float32) * scale,
    }
    out = kernel(**inputs)
    print(out.shape, out.dtype, float(np.abs(out).max()))
